# revision 1
# baseline (speedup 1.0000x reference)
"""Trainium2 Bass kernel for nn_Centroid (segment_reduce + EMA).

Computes, for full inputs:
    sums   = segment_sum(embed, y, C)            # [C, D]
    counts = segment_sum(ones,  y, C)            # [C]
    out    = THETA*centroid + (1-THETA) * sums/(counts+EPS)

Sharding strategy (class-sharded, not batch-sharded):
  Core i owns classes [i*125, (i+1)*125). Host computes, per core, the list
  of batch-row indices whose label is owned by that core (pure index logic).
  Each core then:
    1. gathers its ~B/8 embed rows from HBM via chunked dma_gather (each
       full row is read exactly once across all cores -> same HBM traffic
       as a contiguous batch shard),
    2. builds a local one-hot [128 rows x 128 local classes] per k-tile via
       a host-provided iota constant + is_equal,
    3. matmul-accumulates sums [125,1024] and counts [125,2] in PSUM with
       float32r (full-rate fp32 matmul, TF32-like),
    4. divides by counts, applies the EMA with its centroid slice, and
       writes its 125-row slice of the output.
  No cross-core reduction is needed at all (each class is computed fully on
  one core), so there are no collectives.
"""

import os

import numpy as np

import concourse.bacc as bacc
import concourse.mybir as mybir
import concourse.tile as tile
from concourse import library_config
from concourse.bass_utils import run_bass_kernel_spmd
from concourse.tile_rust import add_dep_helper

NCORES = 8
B = 16384
C = 1000
D = 1024
CPC = C // NCORES  # classes per core = 125
P = 128
THETA = 0.7
EPS = 1e-8
DUMMY = CPC  # local class id used for padding rows; discarded
CHUNK = 512  # rows per dma_gather call

_NC_CACHE: dict[int, object] = {}

# test.py sets KERNEL_TRACE=1 to collect an NTFF profile; results stashed here.
LAST_RESULTS = None


def _build_nc(n_pad: int):
    """Build + compile the per-core Bass program for a padded shard of n_pad rows."""
    f32 = mybir.dt.float32
    bf16 = mybir.dt.bfloat16
    i16 = mybir.dt.int16
    T = n_pad // P  # number of 128-row k-tiles
    # gather chunk sizes (rows), each a multiple of P
    chunks = []
    left = n_pad
    while left > 0:
        c = min(CHUNK, left)
        chunks.append(c)
        left -= c

    nc = bacc.Bacc(
        "TRN2",
        target_bir_lowering=False,
        debug=False,
        enable_asserts=False,
        num_devices=NCORES,
    )
    embed_d = nc.dram_tensor("embed", [B, D], f32, kind="ExternalInput")
    idx_d = nc.dram_tensor("idx", [P, n_pad // 16], i16, kind="ExternalInput")
    yloc_d = nc.dram_tensor("yloc", [P, T], f32, kind="ExternalInput")
    cent_d = nc.dram_tensor("cent", [CPC, D], f32, kind="ExternalInput")
    iota_d = nc.dram_tensor("iotac", [P, P], f32, kind="ExternalInput")
    out_d = nc.dram_tensor("out", [CPC, D], f32, kind="ExternalOutput")

    with tile.TileContext(nc) as tc:
        with (
            tc.tile_pool(name="const", bufs=1) as cpool,
            tc.tile_pool(name="gather", bufs=5) as gpool,
            tc.tile_pool(name="gb", bufs=5) as gbpool,
            tc.tile_pool(name="oh", bufs=4) as ohpool,
            tc.tile_pool(name="psum", bufs=1, space="PSUM") as psum,
        ):
            lib_inst = nc.gpsimd.load_library(library_config.mlp)

            iota_t = cpool.tile([P, P], f32)
            nc.sync.dma_start(out=iota_t[:], in_=iota_d[:])
            ones_t = cpool.tile([P, 2], bf16)
            nc.vector.memset(ones_t[:], 1.0)
            idx_t = cpool.tile([P, n_pad // 16], i16)
            nc.sync.dma_start(out=idx_t[:], in_=idx_d[:])
            yloc_t = cpool.tile([P, T], f32)
            nc.sync.dma_start(out=yloc_t[:], in_=yloc_d[:])
            cent_t = cpool.tile([P, D], f32)
            nc.sync.dma_start(out=cent_t[:CPC, :], in_=cent_d[:])

            ps0 = psum.tile([P, 512], f32)
            ps1 = psum.tile([P, 512], f32)
            pcnt = psum.tile([P, 2], f32)

            t = 0  # global k-tile index
            row0 = 0  # first row of current chunk
            for ch in chunks:
                tiles = ch // P
                g = gpool.tile([P, tiles, D], f32, tag="g")
                gi = nc.gpsimd.dma_gather(
                    g[:],
                    embed_d[:],
                    idx_t[:, row0 // 16 : (row0 + ch) // 16],
                    ch,
                    ch,
                    D,
                )
                add_dep_helper(lib_inst.ins, gi.ins, sync=True, reason="lib before gather")
                gb = gbpool.tile([P, tiles, D], bf16, tag="gb")
                nc.vector.tensor_copy(out=gb[:], in_=g[:])
                for j in range(tiles):
                    oh = ohpool.tile([P, P], bf16, tag="oh")
                    nc.vector.tensor_scalar(
                        out=oh[:],
                        in0=iota_t[:],
                        scalar1=yloc_t[:, t : t + 1],
                        scalar2=None,
                        op0=mybir.AluOpType.is_equal,
                    )
                    st, sp = (t == 0), (t == T - 1)
                    nc.tensor.matmul(
                        ps0[:], lhsT=oh[:], rhs=gb[:, j, 0:512], start=st, stop=sp
                    )
                    nc.tensor.matmul(
                        ps1[:], lhsT=oh[:], rhs=gb[:, j, 512:D], start=st, stop=sp
                    )
                    nc.tensor.matmul(
                        pcnt[:], lhsT=oh[:], rhs=ones_t[:], start=st, stop=sp
                    )
                    t += 1
                row0 += ch

            # inv = (1-THETA) / (counts + EPS)
            inv = cpool.tile([P, 1], f32)
            nc.vector.tensor_scalar(
                out=inv[:],
                in0=pcnt[:, :1],
                scalar1=float(EPS),
                scalar2=None,
                op0=mybir.AluOpType.add,
            )
            nc.vector.reciprocal(inv[:], inv[:])
            nc.vector.tensor_scalar_mul(inv[:], inv[:], float(1.0 - THETA))

            res = cpool.tile([P, D], f32)
            nc.vector.tensor_scalar(
                out=res[:CPC, 0:512],
                in0=ps0[:CPC, :],
                scalar1=inv[:CPC, :1],
                scalar2=None,
                op0=mybir.AluOpType.mult,
            )
            nc.vector.tensor_scalar(
                out=res[:CPC, 512:D],
                in0=ps1[:CPC, :],
                scalar1=inv[:CPC, :1],
                scalar2=None,
                op0=mybir.AluOpType.mult,
            )
            cents = cpool.tile([P, D], f32)
            nc.vector.tensor_scalar_mul(cents[:CPC, :], cent_t[:CPC, :], float(THETA))
            nc.vector.tensor_add(res[:CPC, :], res[:CPC, :], cents[:CPC, :])
            nc.sync.dma_start(out=out_d[:], in_=res[:CPC, :])

    nc.compile()
    return nc


def _shard_inputs(embed: np.ndarray, y: np.ndarray, centroid: np.ndarray):
    """Pure index-side sharding: assign each batch row to its class-owner core."""
    y64 = np.asarray(y).astype(np.int64).ravel()
    owner = y64 // CPC
    order = np.argsort(owner, kind="stable")
    counts = np.bincount(owner, minlength=NCORES)
    n_pad = max(int(-(-counts.max() // P)) * P, P)

    in_maps = []
    start = 0
    T = n_pad // P
    iota = np.broadcast_to(np.arange(P, dtype=np.float32), (P, P)).copy()
    for i in range(NCORES):
        n_i = int(counts[i])
        rows_i = order[start : start + n_i]
        start += n_i
        rows = np.zeros(n_pad, dtype=np.int16)
        rows[:n_i] = rows_i.astype(np.int16)
        yloc = np.full(n_pad, DUMMY, dtype=np.float32)
        yloc[:n_i] = (y64[rows_i] - i * CPC).astype(np.float32)
        # dma_gather idx layout: idx j at [j % 16, j // 16], replicated into
        # all 8 groups of 16 partitions (one copy per gpsimd Q7 core)
        idx_pt = np.tile(rows.reshape(n_pad // 16, 16).T, (8, 1))
        # yloc SBUF layout [128, T]: partition p, col t  <-  flat index t*128+p
        yloc_pt = np.ascontiguousarray(yloc.reshape(T, P).T)
        in_maps.append(
            {
                "embed": embed,
                "idx": idx_pt,
                "yloc": yloc_pt,
                "cent": np.ascontiguousarray(centroid[i * CPC : (i + 1) * CPC]),
                "iotac": iota,
            }
        )
    return in_maps, n_pad


def kernel(embed: np.ndarray, y: np.ndarray, centroid: np.ndarray) -> np.ndarray:
    global LAST_RESULTS
    embed = np.ascontiguousarray(np.asarray(embed, dtype=np.float32))
    centroid = np.ascontiguousarray(np.asarray(centroid, dtype=np.float32))

    in_maps, n_pad = _shard_inputs(embed, y, centroid)
    if n_pad not in _NC_CACHE:
        _NC_CACHE[n_pad] = _build_nc(n_pad)
    nc = _NC_CACHE[n_pad]

    trace = os.environ.get("KERNEL_TRACE", "0") == "1"
    res = run_bass_kernel_spmd(
        nc, in_maps, core_ids=list(range(NCORES)), trace=trace
    )
    LAST_RESULTS = res
    out = np.concatenate([res.results[i]["out"] for i in range(NCORES)], axis=0)
    return out.astype(np.float32)

